# revision 31
# baseline (speedup 1.0000x reference)
"""Decode-phase paged attention with GQA on 8 TRN2 NeuronCores — v4.

Sharding: kv-head axis across the 8 cores (1 kv head + its 4 query heads per
core); q/block_tables/context_lens replicated, metadata baked into the
per-call compiled graph (host resolves the paged gather, device streams every
gathered KV byte from HBM — the memory-bound roofline term).

v4 vs v2/v3:
- KV stream is fp8 e3m4 (half the bytes of v2's bf16; rel err ~1.8e-2 vs
  the 2e-2 gate, dominated by K/V quantization, q stays bf16).
- The PV matmul is FLIPPED: V is the stationary operand (weight loads
  pipeline at ~26ns effective), pt is the 4-column ifmap. The PE's ifmap
  port is column-rate-limited (~0.84 ns/col regardless of dtype), so v2's
  129-column V ifmap stream (107ns/chunk) was the whole critical path.
  Output accumulates per seq as [d=128, G] in PSUM.
- The softmax denominator comes from a per-group ones-weights matmul over
  pt (64 ifmap cols per 16 chunks, amortized ~4ns/chunk) instead of a
  ones column in the V stream; per-fragment extra matmuls cover chunks
  shared by >1 seq (and rmax<128 chunks whose garbage rows would pollute
  the group-wide column sums — the host ignores those main columns).
  Numerator and denominator ship separately; the divide stays on host.
- kv chunk block is [128, 256] (K^T | V), no ones column.
"""

import sys

if "/opt/trn_rl_repo" not in sys.path:
    sys.path.insert(0, "/opt/trn_rl_repo")

import numpy as np
import ml_dtypes

B = 64
H = 32
KVH = 8
G = H // KVH  # 4
D = 128
BS = 16  # tokens per cache block
NB = 8192  # blocks in cache
CH = 128  # tokens per chunk (compute tile)
CW = 2 * D  # kv stream cols per chunk: K(128) | V(128)
QW = B * G + 1  # qt cols: per-seq q heads | ones column (den weights)
ALIGN = 32  # long-seq token padding granularity
SHORT_MAX = 384  # ctx at/below this pads to full chunks (fragment-run guard)
NCHG = 32  # chunks per st/pt group (exp split in halves)
BATCH_CHUNKS = 32  # chunks per KV buffer
SCALE = 0.08838834764831845
NCORES = 8
NEG = -60.0

BF16 = ml_dtypes.bfloat16
FP8 = ml_dtypes.float8_e3m4  # kv stream dtype: 4 mantissa bits, range +-15.5




def _split_frag(r0, lp):
    """Split a (row0, len) piece so each part is a legal tile placement.

    Legal (pos, size): pos 0 any size; pos 32 size<=32; pos 64 size<=64;
    pos 96 size<=32 (round-up sizes 32/64/128 vs 32-aligned positions).
    """
    if r0 == 32 and lp > 32:
        return [(32, 32), (64, lp - 32)]
    return [(r0, lp)]


def plan_problem(block_tables, context_lens):
    bt = np.asarray(block_tables, dtype=np.int64)
    ctx_orig = np.asarray(context_lens, dtype=np.int64)

    # pack shortest-first so the post-last-DMA tail is one long sequence's
    # final chunks -> shortest possible compute/drain chain at the end
    order = np.argsort(ctx_orig, kind="stable")
    ctx = ctx_orig[order]
    bts = bt[order]

    # Every seq pads to whole 128-token chunks. At fp8 the stream is no
    # longer the critical path (the PE weight port is), so the ~4% extra
    # bytes buy zero fragments: single-frag chunks everywhere, no 32-row
    # tile_position fragments (a HW fault class), and one exp per chunk
    # column run instead of per fragment.
    ltok = np.maximum(CH, ((ctx + CH - 1) // CH) * CH)
    starts = np.concatenate([[0], np.cumsum(ltok)[:-1]]).astype(np.int64)
    total = int(ltok.sum())
    nchunk = total // CH
    total128 = nchunk * CH

    # token -> cache-block source (pad/invalid tokens point at block 0 of
    # the owning seq; uncovered gap rows keep blocks=0 and are never read)
    nb = (ctx + BS - 1) // BS
    blocks = np.zeros(total128, dtype=np.int64)
    offs = np.zeros(total128, dtype=np.int64)
    covered = np.zeros(total128, dtype=bool)
    invalid = np.zeros(total128, dtype=bool)
    for p in range(B):
        s0 = int(starts[p])
        L = int(ltok[p])
        loc = np.arange(L)
        blocks[s0 : s0 + L] = bts[p, np.minimum(loc // BS, int(nb[p]) - 1)]
        offs[s0 : s0 + L] = loc % BS
        covered[s0 : s0 + L] = True
        invalid[s0 + int(ctx[p]) : s0 + L] = True

    # per-chunk exp info: rmax (rows covered by fragments — uncovered rows
    # are always a contiguous chunk tail) + bias column for invalid rows
    bias_cols = []
    chunk_bias = [-1] * nchunk
    chunk_rmax = [CH] * nchunk
    for c in range(nchunk):
        cov = covered[c * CH : c * CH + CH]
        rmax = int(np.max(np.nonzero(cov)[0])) + 1 if cov.any() else 0
        assert cov[:rmax].all(), f"non-tail gap in chunk {c}"
        chunk_rmax[c] = rmax
        inv = invalid[c * CH : c * CH + CH]
        if inv[:rmax].any():
            chunk_bias[c] = len(bias_cols)
            bias_cols.append(np.where(inv, NEG, 0.0).astype(np.float32))
    nbias = max(1, len(bias_cols))
    bias_tab = np.zeros((CH, nbias), dtype=np.float32)
    for k, col in enumerate(bias_cols):
        bias_tab[:, k] = col

    # fragments per chunk: (orig_seq, slot, r0, lp, is_first, is_last) —
    # slot p is the drain-order output position, orig seq id indexes qt.
    # Full-chunk padding: exactly one fragment per chunk, rows [0, CH)
    chunk_frags = [[] for _ in range(nchunk)]
    for p in range(B):
        c0 = int(starts[p]) // CH
        ncp = int(ltok[p]) // CH
        for k in range(ncp):
            chunk_frags[c0 + k].append(
                (int(order[p]), p, 0, CH, k == 0, k == ncp - 1)
            )
    assert max(len(f) for f in chunk_frags) == 1

    # batch sizes in chunks: small leading batches so the PE starts ~3us
    # after the first DMA byte instead of waiting a full 32-chunk buffer,
    # bulk 32s after, then 8-chunk tail batches
    sizes = []
    rem = nchunk
    for k in (8, 8, 16):
        if rem <= k + 3:
            break
        sizes.append(k)
        rem -= k
    while rem > 32:
        sizes.append(BATCH_CHUNKS)
        rem -= BATCH_CHUNKS
    while rem > 0:
        k = min(8, rem)
        sizes.append(k)
        rem -= k

    # den layout: one f32 column-group [G] per (chunk, frag). Regular
    # chunks (single frag covering a full-prefix) read the group-wide den
    # matmul's column j; irregular chunks (multi-frag, or rmax<CH whose
    # garbage pt tail rows pollute the group-wide sum) get per-fragment
    # extra columns appended after the group's main block. The host sums
    # each seq's columns; main columns of irregular chunks are ignored.
    den_groups = []  # per group: (g0, gc, colbase, extras=[(j, k, r0, lp)])
    den_col = {}  # (chunk, frag_idx) -> f32 col base in den stream
    colbase = 0
    c0 = 0
    for bc in sizes:
        g0 = c0
        while g0 < c0 + bc:
            gc = min(NCHG, c0 + bc - g0)
            extras = []
            for j in range(gc):
                c = g0 + j
                frs = chunk_frags[c]
                if not frs:
                    continue
                den_col[(c, 0)] = colbase + G * j
                for k in range(1, len(frs)):
                    _, _, r0, lp, _, _ = frs[k]
                    den_col[(c, k)] = colbase + G * (gc + len(extras))
                    extras.append((j, k, r0, lp))
            den_groups.append((g0, gc, colbase, extras))
            colbase += G * (gc + len(extras))
            g0 += gc
        c0 += bc
    nden = colbase
    den_map = [[] for _ in range(B)]  # slot p -> den col bases of its frags
    for c in range(nchunk):
        for k, (_, p, _, _, _, _) in enumerate(chunk_frags[c]):
            den_map[p].append(den_col[(c, k)])

    return {
        "total": total,
        "nchunk": nchunk,
        "batch_sizes": sizes,
        "chunk_frags": chunk_frags,
        "chunk_bias": chunk_bias,
        "chunk_rmax": chunk_rmax,
        "nbias": nbias,
        "bias_tab": bias_tab,
        "blocks": blocks,
        "offs": offs,
        "order": order,
        "den_groups": den_groups,
        "den_map": den_map,
        "nden": nden,
    }


def pack_inputs(plan, q, k_cache, v_cache):
    q = np.asarray(q, dtype=np.float32)
    k_cache = np.asarray(k_cache, dtype=np.float32)
    v_cache = np.asarray(v_cache, dtype=np.float32)

    nchunk = plan["nchunk"]
    blocks, offs = plan["blocks"], plan["offs"]  # [total128], gaps -> block 0

    # one gather for all kv heads: [total128, KVH, D]
    kg = k_cache[blocks, offs]
    vg = v_cache[blocks, offs]

    kv_srcs = []
    for i in range(KVH):
        k3 = kg[:, i, :].reshape(nchunk, CH, D).transpose(0, 2, 1)  # [nc, d, tok]
        v3 = vg[:, i, :].reshape(nchunk, CH, D)  # [nc, tok, d]
        kv3 = np.concatenate([k3, v3], axis=2)  # [nc, 128, 256]
        kv_srcs.append(
            np.ascontiguousarray(kv3.transpose(1, 0, 2).reshape(CH, nchunk * CW)).astype(FP8)
        )

    qs = (q.reshape(B, KVH, G, D) * SCALE).astype(BF16)
    qt = np.ascontiguousarray(qs.transpose(1, 3, 0, 2)).reshape(KVH, D, B * G)
    ones = np.ones((KVH, D, 1), dtype=BF16)
    qt = np.concatenate([qt, ones], axis=2)  # [KVH, D, QW]
    return kv_srcs, qt


def build(plan):
    """Build the (SPMD-identical) Bacc graph for one core."""
    import concourse.mybir as mybir
    import concourse.tile as tile
    from concourse import bacc

    f32 = mybir.dt.float32
    bf16 = mybir.dt.bfloat16
    fp8 = mybir.dt.float8e3
    EXP = mybir.ActivationFunctionType.Exp

    nchunk = plan["nchunk"]
    batch_sizes = plan["batch_sizes"]
    chunk_frags = plan["chunk_frags"]
    chunk_bias = plan["chunk_bias"]
    chunk_rmax = plan["chunk_rmax"]
    den_groups = plan["den_groups"]
    nden = plan["nden"]

    denmax = max(G * (gc + len(ex)) for _, gc, _, ex in den_groups)

    nc = bacc.Bacc()

    kv_ext = nc.declare_dram_parameter("kv", [CH, nchunk * CW], fp8, isOutput=False)
    qt_ext = nc.declare_dram_parameter("qt", [D, QW], bf16, isOutput=False)
    bias_ext = nc.declare_dram_parameter("bias", [CH, plan["nbias"]], f32, isOutput=False)
    out_ext = nc.declare_dram_parameter("out", [CH, B * G], f32, isOutput=True)
    den_ext = nc.declare_dram_parameter("den", [1, nden], f32, isOutput=True)

    with tile.TileContext(nc) as tc:
        with (
            tc.tile_pool(name="const", bufs=1) as const_pool,
            tc.tile_pool(name="kv", bufs=8) as kv_pool,
            tc.tile_pool(name="pt", bufs=4) as pt_pool,
            tc.tile_pool(name="st_psum", bufs=3, space="PSUM") as st_pool,
            tc.tile_pool(name="o_psum", bufs=3, space="PSUM") as o_pool,
            tc.tile_pool(name="den_psum", bufs=2, space="PSUM") as den_pool,
            tc.tile_pool(name="outp", bufs=1) as out_pool,
        ):
            qt_sb = const_pool.tile([D, QW], bf16, name="qt_sb")
            bias_sb = const_pool.tile([CH, plan["nbias"]], f32, name="bias_sb")
            out_sb = out_pool.tile([CH, B * G], f32, name="out_sb")
            den_sb = out_pool.tile([1, nden], f32, name="den_sb")

            o_tiles = {}
            drained = 0
            out_done = 0

            gi = 0
            c0 = 0
            for bi, bc in enumerate(batch_sizes):
                kv_t = kv_pool.tile([CH, CW * bc], fp8, tag="kv", name=f"kv{bi}")
                nc.sync.dma_start(
                    out=kv_t[:, :], in_=kv_ext[:, CW * c0 : CW * (c0 + bc)]
                )
                if bi == 0:
                    # constants after the first kv trigger: kv0's transfer
                    # covers their load, and kv0 starts ~1.2us earlier
                    nc.sync.dma_start(out=qt_sb[:, :], in_=qt_ext[:, :])
                    nc.sync.dma_start(out=bias_sb[:, :], in_=bias_ext[:, :])
                g0 = c0
                while g0 < c0 + bc:
                    gc = min(NCHG, c0 + bc - g0)
                    dg0, dgc, colbase, extras = den_groups[gi]
                    assert dg0 == g0 and dgc == gc
                    gi += 1
                    ncols = gc + len(extras)
                    st_t = st_pool.tile(
                        [CH, G * ncols], f32, tag="st", name=f"st{g0}"
                    )
                    pt_t = pt_pool.tile(
                        [CH, G * ncols], bf16, tag="pt", name=f"pt{g0}"
                    )

                    # tile-column map: fragment 0 of chunk j -> column j;
                    # fragments k>=1 -> private extra columns after gc.
                    # colspec: per column (rows [a,b) covered, bias idx)
                    tilecol = {}
                    colspec = {}
                    ei = 0
                    for j in range(gc):
                        c = g0 + j
                        if not chunk_frags[c]:
                            colspec[j] = (0, 0, -1)
                        for k, (_, _, r0, lp, _, _) in enumerate(chunk_frags[c]):
                            cc = j if k == 0 else gc + ei
                            if k > 0:
                                ei += 1
                            tilecol[(j, k)] = cc
                            colspec[cc] = (r0, r0 + lp, chunk_bias[c])
                    assert ei == len(extras)

                    for c in range(g0, g0 + gc):
                        j = c - g0
                        lc = c - c0
                        for k, (s, p, r0, lp, first, last) in enumerate(
                            chunk_frags[c]
                        ):
                            cc = tilecol[(j, k)]
                            # skip_group_check: each S-matmul is an atomic
                            # start+stop single; CoreSim's zero-region check
                            # is bank-granular and false-positives on two
                            # fragments at different partition offsets
                            nc.tensor.matmul(
                                out=st_t[r0 : r0 + lp, G * cc : G * (cc + 1)],
                                lhsT=kv_t[:, CW * lc + r0 : CW * lc + r0 + lp],
                                rhs=qt_sb[:, G * s : G * (s + 1)],
                                start=True,
                                stop=True,
                                skip_group_check=True,
                                tile_position=(0, r0),
                            )

                    # exps: coalesce runs of plain full columns; partial
                    # columns exp their fragment rows (chunk bias sliced to
                    # the range) and memset the uncovered complement so the
                    # den matmul and its drain only read defined pt bytes
                    half = (gc + 1) // 2
                    run0 = None
                    for cc in range(ncols + 1):
                        sp = colspec.get(cc) if cc < ncols else None
                        plain = sp is not None and sp == (0, CH, -1)
                        if plain:
                            if cc == half and run0 is not None:
                                # split at the half-group boundary: PV of the
                                # first half starts without waiting the rest
                                nc.scalar.activation(
                                    pt_t[:, G * run0 : G * half],
                                    st_t[:, G * run0 : G * half],
                                    EXP,
                                )
                                run0 = cc
                            elif run0 is None:
                                run0 = cc
                            continue
                        if run0 is not None:
                            nc.scalar.activation(
                                pt_t[:, G * run0 : G * cc],
                                st_t[:, G * run0 : G * cc],
                                EXP,
                            )
                            run0 = None
                        if sp is None:
                            continue
                        a, b, bk = sp
                        if a > 0:
                            nc.gpsimd.memset(pt_t[0:a, G * cc : G * (cc + 1)], 0.0)
                        # gpsimd is limited to 32 partitions at nonzero base;
                        # fragment bounds are 32-aligned, so emit 32-row slices
                        for q in range(b, CH, 32):
                            nc.gpsimd.memset(
                                pt_t[q : q + 32, G * cc : G * (cc + 1)], 0.0
                            )
                        if b > a:
                            if bk >= 0:
                                nc.scalar.activation(
                                    pt_t[a:b, G * cc : G * (cc + 1)],
                                    st_t[a:b, G * cc : G * (cc + 1)],
                                    EXP,
                                    bias=bias_sb[a:b, bk : bk + 1],
                                )
                            else:
                                nc.scalar.activation(
                                    pt_t[a:b, G * cc : G * (cc + 1)],
                                    st_t[a:b, G * cc : G * (cc + 1)],
                                    EXP,
                                )

                    # denominator: ONE group-wide ones-weights matmul over
                    # every (chunk, frag) column
                    denw = G * ncols
                    den_t = den_pool.tile([1, denmax], f32, tag="den", name=f"den{g0}")
                    nc.tensor.matmul(
                        out=den_t[0:1, 0:denw],
                        lhsT=qt_sb[:, B * G : QW],
                        rhs=pt_t[:, 0:denw],
                        start=True,
                        stop=True,
                        skip_group_check=True,
                        tile_position=(0, 0),
                    )
                    nc.vector.tensor_copy(
                        out=den_sb[0:1, colbase : colbase + denw],
                        in_=den_t[0:1, 0:denw],
                    )

                    for c in range(g0, g0 + gc):
                        j = c - g0
                        lc = c - c0
                        for k, (s, p, r0, lp, first, last) in enumerate(
                            chunk_frags[c]
                        ):
                            cc = tilecol[(j, k)]
                            if first:
                                o_tiles[p] = o_pool.tile(
                                    [CH, G], f32, tag="o", name=f"o{p}"
                                )
                            nc.tensor.matmul(
                                out=o_tiles[p][:, :],
                                lhsT=kv_t[r0 : r0 + lp, CW * lc + D : CW * lc + CW],
                                rhs=pt_t[r0 : r0 + lp, G * cc : G * (cc + 1)],
                                start=first,
                                stop=last,
                                tile_position=(r0, 0),
                            )
                            if last:
                                # drain slot p: drain order == slot order
                                nc.vector.tensor_copy(
                                    out=out_sb[:, G * p : G * (p + 1)],
                                    in_=o_tiles[p][:, :],
                                )
                                del o_tiles[p]
                                drained += 1
                                if drained in (24, 48):
                                    nc.sync.dma_start(
                                        out=out_ext[:, G * out_done : G * drained],
                                        in_=out_sb[:, G * out_done : G * drained],
                                    )
                                    out_done = drained
                    g0 += gc
                c0 += bc

            nc.sync.dma_start(out=den_ext[:, :], in_=den_sb[:, :])
            nc.sync.dma_start(
                out=out_ext[:, G * out_done :], in_=out_sb[:, G * out_done :]
            )

    nc.compile()
    return nc


def _assemble(results, plan):
    order = plan["order"]
    den_map = plan["den_map"]
    inv = np.argsort(order)  # orig seq -> drain slot
    outs = []
    for i in range(NCORES):
        num = np.asarray(results[i]["out"], dtype=np.float32).reshape(D, B, G)
        den_stream = np.asarray(results[i]["den"], dtype=np.float32).reshape(-1)
        dens = np.zeros((B, G), dtype=np.float32)
        for p in range(B):
            for base in den_map[p]:
                dens[p] += den_stream[base : base + G]
        o = num / dens[None, :, :]  # [D, slot, G]
        outs.append(o[:, inv, :])  # un-permute drain slots back to seq order
    # [KVH][D, B, G] -> [B, KVH, G, D] -> [B, H, D]
    return (
        np.stack(outs, axis=0).transpose(2, 0, 3, 1).reshape(B, H, D).astype(np.float32)
    )


def kernel(q, k_cache, v_cache, block_tables, context_lens, _trace=False):
    from concourse.bass_utils import run_bass_kernel_spmd

    plan = plan_problem(block_tables, context_lens)
    kv_srcs, qt = pack_inputs(plan, q, k_cache, v_cache)
    nc = build(plan)
    in_maps = [
        {
            "kv": kv_srcs[i],
            "qt": qt[i],
            "bias": plan["bias_tab"],
        }
        for i in range(NCORES)
    ]
    res = run_bass_kernel_spmd(nc, in_maps, core_ids=list(range(NCORES)), trace=_trace)
    out = _assemble(res.results, plan)
    if _trace:
        return out, res
    return out


# revision 32
# speedup vs baseline: 1.0854x; 1.0854x over previous
"""Decode-phase paged attention with GQA on 8 TRN2 NeuronCores — v4.

Sharding: kv-head axis across the 8 cores (1 kv head + its 4 query heads per
core); q/block_tables/context_lens replicated, metadata baked into the
per-call compiled graph (host resolves the paged gather, device streams every
gathered KV byte from HBM — the memory-bound roofline term).

v4 vs v2/v3:
- KV stream is fp8 e3m4 (half the bytes of v2's bf16; rel err ~1.8e-2 vs
  the 2e-2 gate, dominated by K/V quantization, q stays bf16).
- The PV matmul is FLIPPED: V is the stationary operand (weight loads
  pipeline at ~26ns effective), pt is the 4-column ifmap. The PE's ifmap
  port is column-rate-limited (~0.84 ns/col regardless of dtype), so v2's
  129-column V ifmap stream (107ns/chunk) was the whole critical path.
  Output accumulates per seq as [d=128, G] in PSUM.
- The softmax denominator comes from a per-group ones-weights matmul over
  pt (64 ifmap cols per 16 chunks, amortized ~4ns/chunk) instead of a
  ones column in the V stream; per-fragment extra matmuls cover chunks
  shared by >1 seq (and rmax<128 chunks whose garbage rows would pollute
  the group-wide column sums — the host ignores those main columns).
  Numerator and denominator ship separately; the divide stays on host.
- kv chunk block is [128, 256] (K^T | V), no ones column.
"""

import sys

if "/opt/trn_rl_repo" not in sys.path:
    sys.path.insert(0, "/opt/trn_rl_repo")

import numpy as np
import ml_dtypes

B = 64
H = 32
KVH = 8
G = H // KVH  # 4
D = 128
BS = 16  # tokens per cache block
NB = 8192  # blocks in cache
CH = 128  # tokens per chunk (compute tile)
CW = 2 * D  # kv stream cols per chunk: K(128) | V(128)
QW = B * G + 1  # qt cols: per-seq q heads | ones column (den weights)
ALIGN = 32  # long-seq token padding granularity
SHORT_MAX = 384  # ctx at/below this pads to full chunks (fragment-run guard)
NCHG = 16  # chunks per st/pt group (exp split in halves)
BATCH_CHUNKS = 32  # chunks per KV buffer
SCALE = 0.08838834764831845
NCORES = 8
NEG = -60.0

BF16 = ml_dtypes.bfloat16
FP8 = ml_dtypes.float8_e3m4  # kv stream dtype: 4 mantissa bits, range +-15.5




def _split_frag(r0, lp):
    """Split a (row0, len) piece so each part is a legal tile placement.

    Legal (pos, size): pos 0 any size; pos 32 size<=32; pos 64 size<=64;
    pos 96 size<=32 (round-up sizes 32/64/128 vs 32-aligned positions).
    """
    if r0 == 32 and lp > 32:
        return [(32, 32), (64, lp - 32)]
    return [(r0, lp)]


def plan_problem(block_tables, context_lens):
    bt = np.asarray(block_tables, dtype=np.int64)
    ctx_orig = np.asarray(context_lens, dtype=np.int64)

    # pack shortest-first so the post-last-DMA tail is one long sequence's
    # final chunks -> shortest possible compute/drain chain at the end
    order = np.argsort(ctx_orig, kind="stable")
    ctx = ctx_orig[order]
    bts = bt[order]

    # Every seq pads to whole 128-token chunks. At fp8 the stream is no
    # longer the critical path (the PE weight port is), so the ~4% extra
    # bytes buy zero fragments: single-frag chunks everywhere, no 32-row
    # tile_position fragments (a HW fault class), and one exp per chunk
    # column run instead of per fragment.
    ltok = np.maximum(CH, ((ctx + CH - 1) // CH) * CH)
    starts = np.concatenate([[0], np.cumsum(ltok)[:-1]]).astype(np.int64)
    total = int(ltok.sum())
    nchunk = total // CH
    total128 = nchunk * CH

    # token -> cache-block source (pad/invalid tokens point at block 0 of
    # the owning seq; uncovered gap rows keep blocks=0 and are never read)
    nb = (ctx + BS - 1) // BS
    blocks = np.zeros(total128, dtype=np.int64)
    offs = np.zeros(total128, dtype=np.int64)
    covered = np.zeros(total128, dtype=bool)
    invalid = np.zeros(total128, dtype=bool)
    for p in range(B):
        s0 = int(starts[p])
        L = int(ltok[p])
        loc = np.arange(L)
        blocks[s0 : s0 + L] = bts[p, np.minimum(loc // BS, int(nb[p]) - 1)]
        offs[s0 : s0 + L] = loc % BS
        covered[s0 : s0 + L] = True
        invalid[s0 + int(ctx[p]) : s0 + L] = True

    # per-chunk exp info: rmax (rows covered by fragments — uncovered rows
    # are always a contiguous chunk tail) + bias column for invalid rows
    bias_cols = []
    chunk_bias = [-1] * nchunk
    chunk_rmax = [CH] * nchunk
    for c in range(nchunk):
        cov = covered[c * CH : c * CH + CH]
        rmax = int(np.max(np.nonzero(cov)[0])) + 1 if cov.any() else 0
        assert cov[:rmax].all(), f"non-tail gap in chunk {c}"
        chunk_rmax[c] = rmax
        inv = invalid[c * CH : c * CH + CH]
        if inv[:rmax].any():
            chunk_bias[c] = len(bias_cols)
            bias_cols.append(np.where(inv, NEG, 0.0).astype(np.float32))
    nbias = max(1, len(bias_cols))
    bias_tab = np.zeros((CH, nbias), dtype=np.float32)
    for k, col in enumerate(bias_cols):
        bias_tab[:, k] = col

    # fragments per chunk: (orig_seq, slot, r0, lp, is_first, is_last) —
    # slot p is the drain-order output position, orig seq id indexes qt.
    # Full-chunk padding: exactly one fragment per chunk, rows [0, CH)
    chunk_frags = [[] for _ in range(nchunk)]
    for p in range(B):
        c0 = int(starts[p]) // CH
        ncp = int(ltok[p]) // CH
        for k in range(ncp):
            chunk_frags[c0 + k].append(
                (int(order[p]), p, 0, CH, k == 0, k == ncp - 1)
            )
    assert max(len(f) for f in chunk_frags) == 1

    # batch sizes in chunks: small leading batches so the PE starts ~3us
    # after the first DMA byte instead of waiting a full 32-chunk buffer,
    # bulk 32s after, then 8-chunk tail batches
    sizes = []
    rem = nchunk
    for k in (8, 8, 16):
        if rem <= k + 3:
            break
        sizes.append(k)
        rem -= k
    while rem > 32:
        sizes.append(BATCH_CHUNKS)
        rem -= BATCH_CHUNKS
    while rem > 0:
        k = min(8, rem)
        sizes.append(k)
        rem -= k

    # den layout: one f32 column-group [G] per (chunk, frag). Regular
    # chunks (single frag covering a full-prefix) read the group-wide den
    # matmul's column j; irregular chunks (multi-frag, or rmax<CH whose
    # garbage pt tail rows pollute the group-wide sum) get per-fragment
    # extra columns appended after the group's main block. The host sums
    # each seq's columns; main columns of irregular chunks are ignored.
    den_groups = []  # per group: (g0, gc, colbase, extras=[(j, k, r0, lp)])
    den_col = {}  # (chunk, frag_idx) -> f32 col base in den stream
    colbase = 0
    c0 = 0
    for bc in sizes:
        g0 = c0
        while g0 < c0 + bc:
            gc = min(NCHG, c0 + bc - g0)
            extras = []
            for j in range(gc):
                c = g0 + j
                frs = chunk_frags[c]
                if not frs:
                    continue
                den_col[(c, 0)] = colbase + G * j
                for k in range(1, len(frs)):
                    _, _, r0, lp, _, _ = frs[k]
                    den_col[(c, k)] = colbase + G * (gc + len(extras))
                    extras.append((j, k, r0, lp))
            den_groups.append((g0, gc, colbase, extras))
            colbase += G * (gc + len(extras))
            g0 += gc
        c0 += bc
    nden = colbase
    den_map = [[] for _ in range(B)]  # slot p -> den col bases of its frags
    for c in range(nchunk):
        for k, (_, p, _, _, _, _) in enumerate(chunk_frags[c]):
            den_map[p].append(den_col[(c, k)])

    return {
        "total": total,
        "nchunk": nchunk,
        "batch_sizes": sizes,
        "chunk_frags": chunk_frags,
        "chunk_bias": chunk_bias,
        "chunk_rmax": chunk_rmax,
        "nbias": nbias,
        "bias_tab": bias_tab,
        "blocks": blocks,
        "offs": offs,
        "order": order,
        "den_groups": den_groups,
        "den_map": den_map,
        "nden": nden,
    }


def pack_inputs(plan, q, k_cache, v_cache):
    q = np.asarray(q, dtype=np.float32)
    k_cache = np.asarray(k_cache, dtype=np.float32)
    v_cache = np.asarray(v_cache, dtype=np.float32)

    nchunk = plan["nchunk"]
    blocks, offs = plan["blocks"], plan["offs"]  # [total128], gaps -> block 0

    # one gather for all kv heads: [total128, KVH, D]
    kg = k_cache[blocks, offs]
    vg = v_cache[blocks, offs]

    kv_srcs = []
    for i in range(KVH):
        k3 = kg[:, i, :].reshape(nchunk, CH, D).transpose(0, 2, 1)  # [nc, d, tok]
        v3 = vg[:, i, :].reshape(nchunk, CH, D)  # [nc, tok, d]
        kv3 = np.concatenate([k3, v3], axis=2)  # [nc, 128, 256]
        kv_srcs.append(
            np.ascontiguousarray(kv3.transpose(1, 0, 2).reshape(CH, nchunk * CW)).astype(FP8)
        )

    qs = (q.reshape(B, KVH, G, D) * SCALE).astype(BF16)
    qt = np.ascontiguousarray(qs.transpose(1, 3, 0, 2)).reshape(KVH, D, B * G)
    ones = np.ones((KVH, D, 1), dtype=BF16)
    qt = np.concatenate([qt, ones], axis=2)  # [KVH, D, QW]
    return kv_srcs, qt


def build(plan):
    """Build the (SPMD-identical) Bacc graph for one core."""
    import concourse.mybir as mybir
    import concourse.tile as tile
    from concourse import bacc

    f32 = mybir.dt.float32
    bf16 = mybir.dt.bfloat16
    fp8 = mybir.dt.float8e3
    EXP = mybir.ActivationFunctionType.Exp

    nchunk = plan["nchunk"]
    batch_sizes = plan["batch_sizes"]
    chunk_frags = plan["chunk_frags"]
    chunk_bias = plan["chunk_bias"]
    chunk_rmax = plan["chunk_rmax"]
    den_groups = plan["den_groups"]
    nden = plan["nden"]

    denmax = max(G * (gc + len(ex)) for _, gc, _, ex in den_groups)

    nc = bacc.Bacc()

    kv_ext = nc.declare_dram_parameter("kv", [CH, nchunk * CW], fp8, isOutput=False)
    qt_ext = nc.declare_dram_parameter("qt", [D, QW], bf16, isOutput=False)
    bias_ext = nc.declare_dram_parameter("bias", [CH, plan["nbias"]], f32, isOutput=False)
    out_ext = nc.declare_dram_parameter("out", [CH, B * G], f32, isOutput=True)
    den_ext = nc.declare_dram_parameter("den", [1, nden], f32, isOutput=True)

    with tile.TileContext(nc) as tc:
        with (
            tc.tile_pool(name="const", bufs=1) as const_pool,
            tc.tile_pool(name="kv", bufs=8) as kv_pool,
            tc.tile_pool(name="pt", bufs=4) as pt_pool,
            tc.tile_pool(name="st_psum", bufs=3, space="PSUM") as st_pool,
            tc.tile_pool(name="o_psum", bufs=3, space="PSUM") as o_pool,
            tc.tile_pool(name="den_psum", bufs=2, space="PSUM") as den_pool,
            tc.tile_pool(name="outp", bufs=1) as out_pool,
        ):
            qt_sb = const_pool.tile([D, QW], bf16, name="qt_sb")
            bias_sb = const_pool.tile([CH, plan["nbias"]], f32, name="bias_sb")
            out_sb = out_pool.tile([CH, B * G], f32, name="out_sb")
            den_sb = out_pool.tile([1, nden], f32, name="den_sb")

            o_tiles = {}
            drained = 0
            out_done = 0

            gi = 0
            c0 = 0
            for bi, bc in enumerate(batch_sizes):
                kv_t = kv_pool.tile([CH, CW * bc], fp8, tag="kv", name=f"kv{bi}")
                nc.sync.dma_start(
                    out=kv_t[:, :], in_=kv_ext[:, CW * c0 : CW * (c0 + bc)]
                )
                if bi == 0:
                    # constants after the first kv trigger: kv0's transfer
                    # covers their load, and kv0 starts ~1.2us earlier
                    nc.sync.dma_start(out=qt_sb[:, :], in_=qt_ext[:, :])
                    nc.sync.dma_start(out=bias_sb[:, :], in_=bias_ext[:, :])
                g0 = c0
                while g0 < c0 + bc:
                    gc = min(NCHG, c0 + bc - g0)
                    dg0, dgc, colbase, extras = den_groups[gi]
                    assert dg0 == g0 and dgc == gc
                    gi += 1
                    ncols = gc + len(extras)
                    st_t = st_pool.tile(
                        [CH, G * ncols], f32, tag="st", name=f"st{g0}"
                    )
                    pt_t = pt_pool.tile(
                        [CH, G * ncols], bf16, tag="pt", name=f"pt{g0}"
                    )

                    # tile-column map: fragment 0 of chunk j -> column j;
                    # fragments k>=1 -> private extra columns after gc.
                    # colspec: per column (rows [a,b) covered, bias idx)
                    tilecol = {}
                    colspec = {}
                    ei = 0
                    for j in range(gc):
                        c = g0 + j
                        if not chunk_frags[c]:
                            colspec[j] = (0, 0, -1)
                        for k, (_, _, r0, lp, _, _) in enumerate(chunk_frags[c]):
                            cc = j if k == 0 else gc + ei
                            if k > 0:
                                ei += 1
                            tilecol[(j, k)] = cc
                            colspec[cc] = (r0, r0 + lp, chunk_bias[c])
                    assert ei == len(extras)

                    for c in range(g0, g0 + gc):
                        j = c - g0
                        lc = c - c0
                        for k, (s, p, r0, lp, first, last) in enumerate(
                            chunk_frags[c]
                        ):
                            cc = tilecol[(j, k)]
                            # skip_group_check: each S-matmul is an atomic
                            # start+stop single; CoreSim's zero-region check
                            # is bank-granular and false-positives on two
                            # fragments at different partition offsets
                            nc.tensor.matmul(
                                out=st_t[r0 : r0 + lp, G * cc : G * (cc + 1)],
                                lhsT=kv_t[:, CW * lc + r0 : CW * lc + r0 + lp],
                                rhs=qt_sb[:, G * s : G * (s + 1)],
                                start=True,
                                stop=True,
                                skip_group_check=True,
                                tile_position=(0, r0),
                            )

                    # exps: coalesce runs of plain full columns; partial
                    # columns exp their fragment rows (chunk bias sliced to
                    # the range) and memset the uncovered complement so the
                    # den matmul and its drain only read defined pt bytes
                    half = (gc + 1) // 2
                    run0 = None
                    for cc in range(ncols + 1):
                        sp = colspec.get(cc) if cc < ncols else None
                        plain = sp is not None and sp == (0, CH, -1)
                        if plain:
                            if cc == half and run0 is not None:
                                # split at the half-group boundary: PV of the
                                # first half starts without waiting the rest
                                nc.scalar.activation(
                                    pt_t[:, G * run0 : G * half],
                                    st_t[:, G * run0 : G * half],
                                    EXP,
                                )
                                run0 = cc
                            elif run0 is None:
                                run0 = cc
                            continue
                        if run0 is not None:
                            nc.scalar.activation(
                                pt_t[:, G * run0 : G * cc],
                                st_t[:, G * run0 : G * cc],
                                EXP,
                            )
                            run0 = None
                        if sp is None:
                            continue
                        a, b, bk = sp
                        if a > 0:
                            nc.gpsimd.memset(pt_t[0:a, G * cc : G * (cc + 1)], 0.0)
                        # gpsimd is limited to 32 partitions at nonzero base;
                        # fragment bounds are 32-aligned, so emit 32-row slices
                        for q in range(b, CH, 32):
                            nc.gpsimd.memset(
                                pt_t[q : q + 32, G * cc : G * (cc + 1)], 0.0
                            )
                        if b > a:
                            if bk >= 0:
                                nc.scalar.activation(
                                    pt_t[a:b, G * cc : G * (cc + 1)],
                                    st_t[a:b, G * cc : G * (cc + 1)],
                                    EXP,
                                    bias=bias_sb[a:b, bk : bk + 1],
                                )
                            else:
                                nc.scalar.activation(
                                    pt_t[a:b, G * cc : G * (cc + 1)],
                                    st_t[a:b, G * cc : G * (cc + 1)],
                                    EXP,
                                )

                    # denominator: ONE group-wide ones-weights matmul over
                    # every (chunk, frag) column
                    denw = G * ncols
                    den_t = den_pool.tile([1, denmax], f32, tag="den", name=f"den{g0}")
                    nc.tensor.matmul(
                        out=den_t[0:1, 0:denw],
                        lhsT=qt_sb[:, B * G : QW],
                        rhs=pt_t[:, 0:denw],
                        start=True,
                        stop=True,
                        skip_group_check=True,
                        tile_position=(0, 0),
                    )
                    nc.vector.tensor_copy(
                        out=den_sb[0:1, colbase : colbase + denw],
                        in_=den_t[0:1, 0:denw],
                    )

                    for c in range(g0, g0 + gc):
                        j = c - g0
                        lc = c - c0
                        for k, (s, p, r0, lp, first, last) in enumerate(
                            chunk_frags[c]
                        ):
                            cc = tilecol[(j, k)]
                            if first:
                                o_tiles[p] = o_pool.tile(
                                    [CH, G], f32, tag="o", name=f"o{p}"
                                )
                            nc.tensor.matmul(
                                out=o_tiles[p][:, :],
                                lhsT=kv_t[r0 : r0 + lp, CW * lc + D : CW * lc + CW],
                                rhs=pt_t[r0 : r0 + lp, G * cc : G * (cc + 1)],
                                start=first,
                                stop=last,
                                tile_position=(r0, 0),
                            )
                            if last:
                                # drain slot p: drain order == slot order
                                nc.vector.tensor_copy(
                                    out=out_sb[:, G * p : G * (p + 1)],
                                    in_=o_tiles[p][:, :],
                                )
                                del o_tiles[p]
                                drained += 1
                                if drained in (24, 48):
                                    nc.sync.dma_start(
                                        out=out_ext[:, G * out_done : G * drained],
                                        in_=out_sb[:, G * out_done : G * drained],
                                    )
                                    out_done = drained
                    g0 += gc
                c0 += bc

            nc.sync.dma_start(out=den_ext[:, :], in_=den_sb[:, :])
            nc.sync.dma_start(
                out=out_ext[:, G * out_done :], in_=out_sb[:, G * out_done :]
            )

    nc.compile()
    return nc


def _assemble(results, plan):
    order = plan["order"]
    den_map = plan["den_map"]
    inv = np.argsort(order)  # orig seq -> drain slot
    outs = []
    for i in range(NCORES):
        num = np.asarray(results[i]["out"], dtype=np.float32).reshape(D, B, G)
        den_stream = np.asarray(results[i]["den"], dtype=np.float32).reshape(-1)
        dens = np.zeros((B, G), dtype=np.float32)
        for p in range(B):
            for base in den_map[p]:
                dens[p] += den_stream[base : base + G]
        o = num / dens[None, :, :]  # [D, slot, G]
        outs.append(o[:, inv, :])  # un-permute drain slots back to seq order
    # [KVH][D, B, G] -> [B, KVH, G, D] -> [B, H, D]
    return (
        np.stack(outs, axis=0).transpose(2, 0, 3, 1).reshape(B, H, D).astype(np.float32)
    )


def kernel(q, k_cache, v_cache, block_tables, context_lens, _trace=False):
    from concourse.bass_utils import run_bass_kernel_spmd

    plan = plan_problem(block_tables, context_lens)
    kv_srcs, qt = pack_inputs(plan, q, k_cache, v_cache)
    nc = build(plan)
    in_maps = [
        {
            "kv": kv_srcs[i],
            "qt": qt[i],
            "bias": plan["bias_tab"],
        }
        for i in range(NCORES)
    ]
    res = run_bass_kernel_spmd(nc, in_maps, core_ids=list(range(NCORES)), trace=_trace)
    out = _assemble(res.results, plan)
    if _trace:
        return out, res
    return out
